# revision 1
# baseline (speedup 1.0000x reference)
"""Trainium2 Bass kernel for nn_Pndb_43344809951805 (scatter_memory).

Data-parallel over batch B=16 across 8 NeuronCores (2 batches/core).
Phase 1 writes the [Q,D] memory A (mean over B -> AllReduce), phase 2 reads it.
All big matmuls run in bf16 (full PE rate); residual path stays f32.
"""
import sys

sys.path.insert(0, "/opt/trn_rl_repo")

import numpy as np
import ml_dtypes

import concourse.bass as bass
import concourse.bacc as bacc
import concourse.mybir as mybir
import concourse.tile as tile
from concourse import masks
from concourse.bass_utils import run_bass_kernel_spmd

F32 = mybir.dt.float32
BF16 = mybir.dt.bfloat16
AF = mybir.ActivationFunctionType
ALU = mybir.AluOpType
BF = ml_dtypes.bfloat16

B, S, D, Q = 16, 2048, 1024, 64
NCORES = 8
BL = B // NCORES          # local batches per core
SBLK = 512                # s-block (matmul moving free dim)
NSB = S // SBLK           # 4 s-blocks per batch
NCH = S // 128            # 16 s-chunks per batch
NJ = D // 128             # 8 contraction chunks
NI = D // 128             # 8 output-dim chunks
CPB = SBLK // 128         # 4 chunks per s-block

_prog_cache = {}


def _build(bi_v: float, cgate_v: float, stage: str = "full"):
    nc = bacc.Bacc("TRN2", target_bir_lowering=False, debug=False,
                   enable_asserts=False, num_devices=NCORES)

    rawT_d = nc.dram_tensor("rawT", [BL, D, S], BF16, kind="ExternalInput")
    rawN_d = nc.dram_tensor("rawN", [BL, S, D], BF16, kind="ExternalInput")
    pdT_d = nc.dram_tensor("pdT", [BL, D, S], BF16, kind="ExternalInput")
    pdN_d = nc.dram_tensor("pdN", [BL, S, D], F32, kind="ExternalInput")
    wkT_d = nc.dram_tensor("wkT", [D, D], BF16, kind="ExternalInput")
    woT_d = nc.dram_tensor("woT", [D, D], BF16, kind="ExternalInput")
    qT1_d = nc.dram_tensor("qT1", [D, Q], BF16, kind="ExternalInput")
    qT2_d = nc.dram_tensor("qT2", [D, Q], BF16, kind="ExternalInput")
    bkT_d = nc.dram_tensor("bkT", [D, 1], F32, kind="ExternalInput")
    boT_d = nc.dram_tensor("boT", [D, 1], F32, kind="ExternalInput")
    wiB_d = nc.dram_tensor("wiB", [128, D], BF16, kind="ExternalInput")
    wu1B_d = nc.dram_tensor("wu1B", [128, D], F32, kind="ExternalInput")
    wu2B_d = nc.dram_tensor("wu2B", [Q, D], F32, kind="ExternalInput")
    out_d = nc.dram_tensor("out", [BL, S, D], F32, kind="ExternalOutput")

    with tile.TileContext(nc) as tc:
        with (
            tc.tile_pool(name="const", bufs=1) as cp,
            tc.tile_pool(name="dram", bufs=1, space="DRAM") as dram,
        ):
            # ---- constants; only wkT DMA'd up-front (first-MM critical) ----
            ident = cp.tile([128, 128], BF16, tag="ident")
            masks.make_identity(nc, ident[:])
            nbi = cp.tile([128, 1], F32, tag="nbi")
            nc.vector.memset(nbi[:], -bi_v)
            ncg = cp.tile([128, 1], F32, tag="ncg")
            nc.vector.memset(ncg[:], -cgate_v)

            wkT = [cp.tile([128, D], BF16, tag=f"wkT{j}", name=f"wkT{j}")
                   for j in range(NJ)]
            woT = [cp.tile([128, D], BF16, tag=f"woT{j}", name=f"woT{j}")
                   for j in range(NJ)]
            qT1 = [cp.tile([128, Q], BF16, tag=f"qT1{j}", name=f"qT1{j}")
                   for j in range(NJ)]
            qT2 = [cp.tile([128, Q], BF16, tag=f"qT2{j}", name=f"qT2{j}")
                   for j in range(NJ)]
            bkT = [cp.tile([128, 1], F32, tag=f"bkT{j}", name=f"bkT{j}")
                   for j in range(NJ)]
            boT = [cp.tile([128, 1], F32, tag=f"boT{j}", name=f"boT{j}")
                   for j in range(NJ)]
            wiB = cp.tile([128, D], BF16, tag="wiB")
            wu1B = cp.tile([128, D], F32, tag="wu1B")
            wu2B = cp.tile([Q, D], F32, tag="wu2B")
            for j in range(NJ):
                nc.sync.dma_start(wkT[j][:],
                                  wkT_d[j * 128:(j + 1) * 128, :])

            A_acc = cp.tile([Q, D], F32, tag="A_acc")
            A_f32 = cp.tile([Q, D], F32, tag="A_f32")
            A_bf = cp.tile([Q, D], BF16, tag="A_bf")
            awB = cp.tile([128, Q], BF16, tag="awB")
            scrA = cp.tile([Q, D], F32, tag="scrA")
            aw = cp.tile([Q, 1], F32, tag="aw")
            ar_in = dram.tile([Q + 1, D + 8], BF16)
            ar_out = dram.tile([Q + 1, D + 8], BF16)
            awz = cp.tile([Q, 8], BF16, tag="awz")
            nc.vector.memset(awz[:], 0.0)
            zrow = cp.tile([1, D + 8], BF16, tag="zrow")
            nc.vector.memset(zrow[:], 0.0)
            nc.gpsimd.dma_start(ar_in[0:Q, D:D + 8], awz[:])
            nc.gpsimd.dma_start(ar_in[Q:Q + 1, Q:D + 8], zrow[:, Q:D + 8])
            # phase-2 first-block data, prefetched during phase 1
            pdt0 = [cp.tile([128, SBLK], BF16, tag=f"pdt0_{j}",
                            name=f"pdt0_{j}") for j in range(NJ)]
            pdn0 = [cp.tile([128, D], F32, tag=f"pdn0_{c}",
                            name=f"pdn0_{c}") for c in range(CPB)]


            # ================= PHASE 1 =================
            with (
                tc.tile_pool(name="p1", bufs=1) as p1,
                tc.tile_pool(name="p1ps", bufs=1, space="PSUM") as p1ps,
            ):
                def load_rawt(b, sb):
                    ts = []
                    for j in range(NJ):
                        t = p1.tile([128, SBLK], BF16, tag=f"rawt{j}",
                                    name=f"rawt{j}", bufs=2)
                        nc.sync.dma_start(
                            t[:], rawT_d[b, j * 128:(j + 1) * 128,
                                         sb * SBLK:(sb + 1) * SBLK])
                        ts.append(t)
                    return ts

                for b in range(BL):
                    rawt = load_rawt(b, 0)
                    if b == 0:
                        nc.sync.dma_start(wiB[:], wiB_d[:])
                        for j in range(NJ):
                            sl = slice(j * 128, (j + 1) * 128)
                            nc.sync.dma_start(bkT[j][:], bkT_d[sl, :])
                            nc.sync.dma_start(qT1[j][:], qT1_d[sl, :])
                    U = p1.tile([Q, S], BF16, tag="U", bufs=2)
                    Zp = p1.tile([Q, NSB], F32, tag="Zp", bufs=2)
                    A_ps = p1ps.tile([Q, D], F32, tag="A_ps", bufs=1)

                    for sb in range(NSB):
                        # v-gate pre-pass + v for this s-block's chunks
                        Gg = p1.tile([128, CPB], F32, tag="Gg", bufs=2)
                        rns = []
                        for cc in range(CPB):
                            c = sb * CPB + cc
                            rn = p1.tile([128, D], BF16, tag=f"rawn{cc}",
                                         name=f"rawn{cc}", bufs=2)
                            nc.sync.dma_start(
                                rn[:], rawN_d[b, c * 128:(c + 1) * 128, :])
                            rns.append(rn)
                            scr = p1.tile([128, D], BF16, tag="scrb", bufs=2)
                            nc.vector.scalar_tensor_tensor(
                                scr[:], rn[:], 1.0, wiB[:],
                                ALU.mult, ALU.mult,
                                accum_out=Gg[:, cc:cc + 1])
                        nc.scalar.activation(Gg[:], Gg[:], AF.Exp,
                                             scale=-1.0, bias=nbi[:])
                        nc.vector.tensor_scalar_add(Gg[:], Gg[:], 1.0)
                        nc.vector.reciprocal(Gg[:], Gg[:])
                        vs = []
                        for cc in range(CPB):
                            v = p1.tile([128, D], BF16, tag=f"v{cc}",
                                        name=f"v{cc}", bufs=2)
                            nc.vector.tensor_scalar_mul(
                                v[:], rns[cc][:], Gg[:, cc:cc + 1])
                            vs.append(v)

                        nxt = load_rawt(b, sb + 1) if sb + 1 < NSB else None
                        if b == 0 and sb == 0:
                            # phase-2 weights: off the critical DMA path
                            for j in range(NJ):
                                sl = slice(j * 128, (j + 1) * 128)
                                nc.sync.dma_start(woT[j][:], woT_d[sl, :])
                                nc.sync.dma_start(qT2[j][:], qT2_d[sl, :])
                                nc.sync.dma_start(boT[j][:], boT_d[sl, :])
                            nc.sync.dma_start(wu1B[:], wu1B_d[:])
                            nc.sync.dma_start(wu2B[:], wu2B_d[:])
                        if b == 1 and sb == 0:
                            # prefetch phase-2 (b0, sb0) data
                            for j in range(NJ):
                                nc.sync.dma_start(
                                    pdt0[j][:],
                                    pdT_d[0, j * 128:(j + 1) * 128, 0:SBLK])
                            for c in range(CPB):
                                nc.sync.dma_start(
                                    pdn0[c][:],
                                    pdN_d[0, c * 128:(c + 1) * 128, :])

                        # kT matmuls + scores (software-pipelined by one i)
                        sc_ps = p1ps.tile([Q, SBLK], F32, tag="sc_ps", bufs=2)
                        kts = []
                        for i in range(NI):
                            isl = slice(i * 128, (i + 1) * 128)
                            k_ps = p1ps.tile([128, SBLK], F32, tag="k_ps",
                                             bufs=2)
                            for j in range(NJ):
                                nc.tensor.matmul(
                                    k_ps[:], wkT[j][:, isl], rawt[j][:],
                                    start=(j == 0), stop=(j == NJ - 1))
                            kt = p1.tile([128, SBLK], BF16, tag="kt", bufs=3)
                            nc.scalar.activation(kt[:], k_ps[:], AF.Identity,
                                                 bias=bkT[i][:])
                            kts.append(kt)
                            if i >= 1:
                                nc.tensor.matmul(
                                    sc_ps[:], qT1[i - 1][:], kts[i - 1][:],
                                    start=(i - 1 == 0), stop=False,
                                    skip_group_check=True)
                        nc.tensor.matmul(
                            sc_ps[:], qT1[NI - 1][:], kts[NI - 1][:],
                            start=False, stop=True, skip_group_check=True)

                        ssl = slice(sb * SBLK, (sb + 1) * SBLK)
                        nc.scalar.activation(U[:, ssl], sc_ps[:], AF.Exp,
                                             accum_out=Zp[:, sb:sb + 1])
                        # transposes first (decoupled from A matmuls)
                        uts = []
                        for cc in range(CPB):
                            c = sb * CPB + cc
                            ut_ps = p1ps.tile([128, Q], BF16, tag="ut_ps",
                                              bufs=2)
                            nc.tensor.transpose(
                                ut_ps[:], U[:, c * 128:(c + 1) * 128],
                                ident[:Q, :Q])
                            ut = p1.tile([128, Q], BF16, tag="ut", bufs=6)
                            nc.scalar.copy(ut[:], ut_ps[:])
                            uts.append(ut)
                        for cc in range(CPB):
                            c = sb * CPB + cc
                            for h in range(2):
                                hsl = slice(h * 512, (h + 1) * 512)
                                nc.tensor.matmul(
                                    A_ps[:, hsl], uts[cc][:], vs[cc][:, hsl],
                                    start=(c == 0), stop=(c == NCH - 1),
                                    skip_group_check=True)
                        rawt = nxt

                    # A_acc += A_ps / (16 * Z)
                    Z1 = p1.tile([Q, 1], F32, tag="Z1", bufs=2)
                    nc.vector.tensor_reduce(Z1[:], Zp[:], mybir.AxisListType.X,
                                            ALU.add)
                    sA = p1.tile([Q, 1], F32, tag="sA", bufs=2)
                    nc.vector.reciprocal(sA[:], Z1[:])
                    nc.vector.tensor_scalar_mul(sA[:], sA[:], 1.0 / B)
                    if b == 0:
                        nc.vector.tensor_scalar_mul(A_acc[:], A_ps[:], sA[:])
                    else:
                        nc.vector.scalar_tensor_tensor(
                            A_acc[:], A_ps[:], sA[:], A_acc[:],
                            ALU.mult, ALU.add)

                # aw_local = A_acc . Wu2 rides along in the AllReduce
                nc.vector.scalar_tensor_tensor(
                    scrA[:], A_acc[:], 1.0, wu2B[:],
                    ALU.mult, ALU.mult, accum_out=aw[:])
                nc.gpsimd.dma_start(ar_in[0:Q, 0:D], A_acc[:])
                nc.gpsimd.dma_start(
                    ar_in[Q:Q + 1, 0:Q].rearrange("a b -> b a"), aw[:])

            # ---- AllReduce of partial A across the 8 cores ----
            if stage == "p1":
                nc.sync.dma_start(out_d[0, 0:Q, :], A_acc[:])
            else:
                if stage == "p2":
                    arr = ar_in
                else:
                    nc.gpsimd.collective_compute(
                        "AllReduce", ALU.add,
                        replica_groups=[list(range(NCORES))],
                        ins=[ar_in.opt()], outs=[ar_out.opt()],
                    )
                    arr = ar_out
                if stage == "ar":
                    nc.gpsimd.dma_start(A_f32[:], arr[0:Q, 0:D])
                nc.gpsimd.dma_start(A_bf[:], arr[0:Q, 0:D])
                nc.gpsimd.dma_start(
                    awB[:], arr[Q:Q + 1, 0:Q].broadcast_to([128, Q]))

            # ================= PHASE 2 =================
            if stage == "p1":
                pass  # skip phase 2
            else:
              with (
                  tc.tile_pool(name="p2", bufs=1) as p2,
                  tc.tile_pool(name="p2ps", bufs=1, space="PSUM") as p2ps,
              ):
                  def load_pdt(b, sb):
                      ts = []
                      for j in range(NJ):
                          t = p2.tile([128, SBLK], BF16, tag=f"pdt{j}",
                                      name=f"pdt{j}", bufs=2)
                          nc.sync.dma_start(
                              t[:], pdT_d[b, j * 128:(j + 1) * 128,
                                          sb * SBLK:(sb + 1) * SBLK])
                          ts.append(t)
                      return ts

                  def emit_ko(pdt):
                      kot = []
                      for i in range(NI):
                          isl = slice(i * 128, (i + 1) * 128)
                          ko_ps = p2ps.tile([128, SBLK], F32, tag="ko_ps",
                                            bufs=2)
                          for j in range(NJ):
                              nc.tensor.matmul(
                                  ko_ps[:], woT[j][:, isl], pdt[j][:],
                                  start=(j == 0), stop=(j == NJ - 1))
                          kt = p2.tile([128, SBLK], BF16, tag="kot",
                                       name="kot", bufs=16)
                          nc.scalar.activation(kt[:], ko_ps[:], AF.Identity,
                                               bias=boT[i][:])
                          kot.append(kt)
                      return kot

                  def emit_partB(st):
                      (Z2, u2s, b, sb, idx, G1, pdn) = st
                      G2 = p2.tile([128, CPB], F32, tag="G2", bufs=2)
                      SC = p2.tile([128, CPB], F32, tag="SC", bufs=2)
                      if G1 is None:
                          G1 = p2.tile([128, CPB], F32, tag="G1", bufs=2)
                          pdn = []
                          for cc in range(CPB):
                              c = sb * CPB + cc
                              if idx == 0:
                                  pn = pdn0[cc]
                              else:
                                  pn = p2.tile([128, D], F32,
                                               tag=f"pdn{cc}",
                                               name=f"pdn{cc}", bufs=2)
                                  nc.sync.dma_start(
                                      pn[:],
                                      pdN_d[b, c * 128:(c + 1) * 128, :])
                              pdn.append(pn)
                              scr = p2.tile([128, D], F32, tag="scrf",
                                            bufs=2)
                              nc.vector.scalar_tensor_tensor(
                                  scr[:], pn[:], 1.0, wu1B[:],
                                  ALU.mult, ALU.mult,
                                  accum_out=G1[:, cc:cc + 1])
                      for cc in range(CPB):
                          scr2 = p2.tile([128, Q], BF16, tag="scr2",
                                         bufs=2)
                          nc.vector.scalar_tensor_tensor(
                              scr2[:], u2s[cc][:], 1.0, awB[:],
                              ALU.mult, ALU.mult,
                              accum_out=G2[:, cc:cc + 1])
                      # gates: sc = sigmoid(g1 + g2/Z + cg) / Z
                      rz = p2.tile([128, CPB], F32, tag="rz", bufs=2)
                      nc.vector.reciprocal(rz[:], Z2[:])
                      t4 = p2.tile([128, CPB], F32, tag="t4", bufs=2)
                      nc.vector.tensor_mul(t4[:], G2[:], rz[:])
                      nc.vector.tensor_add(t4[:], t4[:], G1[:])
                      e4 = p2.tile([128, CPB], F32, tag="e4", bufs=2)
                      nc.scalar.activation(e4[:], t4[:], AF.Exp,
                                           scale=-1.0, bias=ncg[:])
                      nc.vector.tensor_scalar_add(e4[:], e4[:], 1.0)
                      nc.vector.reciprocal(e4[:], e4[:])
                      nc.vector.tensor_mul(SC[:], e4[:], rz[:])
                      # transposes, then A2 matmuls + fused residual
                      ut2s = []
                      for cc in range(CPB):
                          ut2_ps = p2ps.tile([Q, 128], BF16, tag="ut2_ps",
                                             bufs=2)
                          nc.tensor.transpose(ut2_ps[:], u2s[cc][:],
                                              ident[:, :])
                          ut2 = p2.tile([Q, 128], BF16, tag="ut2", bufs=6)
                          nc.scalar.copy(ut2[:], ut2_ps[:])
                          ut2s.append(ut2)
                      for cc in range(CPB):
                          c = sb * CPB + cc
                          outt = p2.tile([128, D], F32, tag="outt", bufs=3)
                          for h in range(2):
                              hsl = slice(h * 512, (h + 1) * 512)
                              a2_ps = p2ps.tile([128, 512], F32,
                                                tag="a2_ps", bufs=2)
                              nc.tensor.matmul(a2_ps[:], ut2s[cc][:],
                                               A_bf[:, hsl],
                                               start=True, stop=True)
                              nc.vector.scalar_tensor_tensor(
                                  outt[:, hsl], a2_ps[:],
                                  SC[:, cc:cc + 1],
                                  pdn[cc][:, hsl], ALU.mult, ALU.add)
                          nc.sync.dma_start(
                              out_d[b, c * 128:(c + 1) * 128, :], outt[:])

                  all_sb = [(b, sb) for b in range(BL) for sb in range(NSB)]
                  pdt_cur = pdt0
                  pending = []
                  for idx, (b, sb) in enumerate(all_sb):
                      kot = emit_ko(pdt_cur)
                      pdt_nxt = (load_pdt(*all_sb[idx + 1])
                                 if idx + 1 < len(all_sb) else None)
                      # part A: s2 + exp per chunk (A-independent)
                      Z2 = p2.tile([128, CPB], F32, tag="Z2", bufs=4)
                      u2s = []
                      for cc in range(CPB):
                          c = sb * CPB + cc
                          s2_ps = p2ps.tile([128, Q], F32, tag="s2_ps",
                                            bufs=2)
                          for i in range(NI):
                              nc.tensor.matmul(
                                  s2_ps[:],
                                  kot[i][:, cc * 128:(cc + 1) * 128],
                                  qT2[i][:],
                                  start=(i == 0), stop=(i == NI - 1),
                                  skip_group_check=True)
                          u2 = p2.tile([128, Q], BF16, tag=f"u2{cc}",
                                       name=f"u2{cc}", bufs=4)
                          nc.scalar.activation(u2[:], s2_ps[:], AF.Exp,
                                               accum_out=Z2[:, cc:cc + 1])
                          u2s.append(u2)

                      G1e = None
                      pdne = None
                      if idx >= len(all_sb) - 2:
                          G1e = p2.tile([128, CPB], F32, tag="G1L", bufs=2)
                          pdne = []
                          for cc in range(CPB):
                              c = sb * CPB + cc
                              pn = p2.tile([128, D], F32, tag=f"pdnL{cc}",
                                           name=f"pdnL{cc}", bufs=2)
                              nc.sync.dma_start(
                                  pn[:],
                                  pdN_d[b, c * 128:(c + 1) * 128, :])
                              pdne.append(pn)
                              scr = p2.tile([128, D], F32, tag="scrf",
                                            bufs=2)
                              nc.vector.scalar_tensor_tensor(
                                  scr[:], pn[:], 1.0, wu1B[:],
                                  ALU.mult, ALU.mult,
                                  accum_out=G1e[:, cc:cc + 1])

                      depth = 3 if idx <= 4 else 2
                      while len(pending) >= depth:
                          emit_partB(pending.pop(0))
                      pending.append((Z2, u2s, b, sb, idx, G1e, pdne))
                      pdt_cur = pdt_nxt
                  for st in pending:
                      emit_partB(st)

            if stage == "ar":
                nc.sync.dma_start(out_d[0, 0:Q, :], A_f32[:])
                nc.gpsimd.dma_start(out_d[0, 128:256, 0:Q], awB[:])
    nc.compile()
    return nc


def _get_prog(bi_v, cgate_v):
    key = (round(bi_v, 9), round(cgate_v, 9))
    if key not in _prog_cache:
        _prog_cache[key] = _build(bi_v, cgate_v)
    return _prog_cache[key]


def kernel(raw, post_dec, mask, questions, Wk, bk, Wi, bi, Wo, bo,
           Wu1, bu1, Wu2, bu2, b1, _trace=False):
    raw = np.asarray(raw, dtype=np.float32)
    post_dec = np.asarray(post_dec, dtype=np.float32)
    questions = np.asarray(questions, dtype=np.float32)
    Wk = np.asarray(Wk, dtype=np.float32)
    Wo = np.asarray(Wo, dtype=np.float32)

    bi_v = float(np.asarray(bi).reshape(-1)[0])
    cgate_v = float(np.asarray(bu1).reshape(-1)[0]
                    + np.asarray(bu2).reshape(-1)[0]
                    + np.asarray(b1).reshape(-1)[0])
    nc = _get_prog(bi_v, cgate_v)

    inv_sqrt_d = np.float32(1.0 / np.sqrt(D))
    inv_sqrt_q = np.float32(1.0 / np.sqrt(Q))
    wkT = np.ascontiguousarray(Wk.T).astype(BF)
    woT = np.ascontiguousarray(Wo.T).astype(BF)
    qT1 = np.ascontiguousarray(questions.T * inv_sqrt_d).astype(BF)
    qT2 = np.ascontiguousarray(questions.T * inv_sqrt_q).astype(BF)
    bkT = np.ascontiguousarray(np.asarray(bk, np.float32).reshape(D, 1))
    boT = np.ascontiguousarray(np.asarray(bo, np.float32).reshape(D, 1))
    wiB = np.ascontiguousarray(
        np.broadcast_to(np.asarray(Wi, np.float32).reshape(1, D), (128, D))
    ).astype(BF)
    wu1B = np.ascontiguousarray(
        np.broadcast_to(np.asarray(Wu1, np.float32).reshape(1, D), (128, D)))
    wu2B = np.ascontiguousarray(
        np.broadcast_to(np.asarray(Wu2, np.float32).reshape(1, D), (Q, D)))

    in_maps = []
    for r in range(NCORES):
        bs = slice(r * BL, (r + 1) * BL)
        rawT = np.ascontiguousarray(
            raw[bs].transpose(0, 2, 1)).astype(BF)
        rawN = np.ascontiguousarray(raw[bs]).astype(BF)
        pdT = np.ascontiguousarray(
            post_dec[bs].transpose(0, 2, 1)).astype(BF)
        pdN = np.ascontiguousarray(post_dec[bs])
        in_maps.append({
            "rawT": rawT, "rawN": rawN, "pdT": pdT, "pdN": pdN,
            "wkT": wkT, "woT": woT, "qT1": qT1, "qT2": qT2,
            "bkT": bkT, "boT": boT, "wiB": wiB, "wu1B": wu1B, "wu2B": wu2B,
        })

    res = run_bass_kernel_spmd(nc, in_maps, core_ids=list(range(NCORES)),
                               trace=_trace)
    out = np.concatenate([res.results[r]["out"] for r in range(NCORES)],
                         axis=0)
    if _trace:
        kernel._last_result = res
    return out



# revision 6
# speedup vs baseline: 1.7323x; 1.7323x over previous
"""Trainium2 Bass kernel for nn_Pndb_43344809951805 (scatter_memory).

Data-parallel over batch B=16 across 8 NeuronCores (2 batches/core).

Key algebraic rewrite vs the reference: both [S,D]x[D,D] projections are
reassociated away.
  Phase 1: scores = (questions @ Wk) @ raw^T  (the q.bk bias is a per-row
           constant, softmax-invariant over s). The sigmoid gate on v is
           rank-1, folded into the attn rows after the PE transpose.
  Phase 2: s2 = pd @ (Wo^T @ q^T) + (bo.q)^T (rank-1 bias rides the PSUM
           accumulation as a 1-partition matmul).
This drops PE work ~10x; the kernel is then HBM-bound (~42 MB/core).
"""
import sys

sys.path.insert(0, "/opt/trn_rl_repo")

import numpy as np
import ml_dtypes

import concourse.bass as bass
import concourse.bacc as bacc
import concourse.mybir as mybir
import concourse.tile as tile
from concourse import masks
from concourse.bass_utils import run_bass_kernel_spmd

F32 = mybir.dt.float32
BF16 = mybir.dt.bfloat16
AF = mybir.ActivationFunctionType
ALU = mybir.AluOpType
BF = ml_dtypes.bfloat16

B, S, D, Q = 16, 2048, 1024, 64
NCORES = 8
BL = B // NCORES          # local batches per core
SBLK = 512                # s-block
NSB = S // SBLK           # 4 s-blocks per batch
NCH = S // 128            # 16 s-chunks per batch
NJ = D // 128             # 8 contraction chunks
CPB = SBLK // 128         # 4 chunks per s-block

_prog_cache = {}


def _build(bi_v: float, cgate_v: float):
    nc = bacc.Bacc("TRN2", target_bir_lowering=False, debug=False,
                   enable_asserts=False, num_devices=NCORES)

    rawT_d = nc.dram_tensor("rawT", [BL, D, S], BF16, kind="ExternalInput")
    rawN_d = nc.dram_tensor("rawN", [BL, S, D], BF16, kind="ExternalInput")
    pdT_d = nc.dram_tensor("pdT", [BL, D, S], BF16, kind="ExternalInput")
    pdN_d = nc.dram_tensor("pdN", [BL, S, D], BF16, kind="ExternalInput")
    qkT_d = nc.dram_tensor("qkT", [D, Q], BF16, kind="ExternalInput")
    woq_d = nc.dram_tensor("woq", [D, Q], BF16, kind="ExternalInput")
    boq_d = nc.dram_tensor("boq", [1, Q], BF16, kind="ExternalInput")
    wiB_d = nc.dram_tensor("wiB", [128, D], BF16, kind="ExternalInput")
    wu1B_d = nc.dram_tensor("wu1B", [128, D], BF16, kind="ExternalInput")
    wu2B_d = nc.dram_tensor("wu2B", [Q, D], F32, kind="ExternalInput")
    out_d = nc.dram_tensor("out", [BL, S, D], BF16, kind="ExternalOutput")

    def dma_blk(tile_, dram, b, sb, store=False):
        # [512, 1024] DRAM block <-> [128, 4*1024] SBUF tile, 4 DMAs
        for cc in range(CPB):
            c = sb * CPB + cc
            dsl = dram[b, c * 128:(c + 1) * 128, :]
            ssl = tile_[:, cc * D:(cc + 1) * D]
            if store:
                nc.sync.dma_start(dsl, ssl)
            else:
                nc.sync.dma_start(ssl, dsl)

    with tile.TileContext(nc) as tc:
        with (
            tc.tile_pool(name="const", bufs=1) as cp,
            tc.tile_pool(name="dram", bufs=1, space="DRAM") as dram,
        ):
            ident = cp.tile([128, 128], BF16, tag="ident")
            masks.make_identity(nc, ident[:])
            nbi = cp.tile([128, 1], F32, tag="nbi")
            nc.vector.memset(nbi[:], -bi_v)
            ncg = cp.tile([128, 1], F32, tag="ncg")
            nc.vector.memset(ncg[:], -cgate_v)
            ones1 = cp.tile([1, 128], BF16, tag="ones1")
            nc.vector.memset(ones1[:], 1.0)

            qkT = [cp.tile([128, Q], BF16, tag=f"qkT{j}", name=f"qkT{j}")
                   for j in range(NJ)]
            woq = [cp.tile([128, Q], BF16, tag=f"woq{j}", name=f"woq{j}")
                   for j in range(NJ)]
            boqT = cp.tile([1, Q], BF16, tag="boqT")
            wiB = cp.tile([128, D], BF16, tag="wiB")
            wu1B = cp.tile([128, D], BF16, tag="wu1B")
            wu2B = cp.tile([Q, D], F32, tag="wu2B")

            A_acc = cp.tile([Q, D], F32, tag="A_acc")
            A_bf = cp.tile([Q, D], BF16, tag="A_bf")
            awB = cp.tile([128, Q], BF16, tag="awB")
            scrA = cp.tile([Q, D], F32, tag="scrA")
            aw = cp.tile([Q, 1], F32, tag="aw")
            ar_in = dram.tile([Q + 1, D + 8], BF16)
            ar_out = dram.tile([Q + 1, D + 8], BF16)
            awz = cp.tile([Q, 8], BF16, tag="awz")
            nc.vector.memset(awz[:], 0.0)
            zrow = cp.tile([1, D + 8], BF16, tag="zrow")
            nc.vector.memset(zrow[:], 0.0)
            nc.gpsimd.dma_start(ar_in[0:Q, D:D + 8], awz[:])
            nc.gpsimd.dma_start(ar_in[Q:Q + 1, Q:D + 8], zrow[:, Q:D + 8])
            # phase-2 b0 pdT + block-0/1 pdN, prefetched late in phase 1
            pdt0 = [cp.tile([128, S], BF16, tag=f"pdt0_{j}",
                            name=f"pdt0_{j}") for j in range(NJ)]

            # ================= PHASE 1 =================
            with (
                tc.tile_pool(name="p1", bufs=1) as p1,
                tc.tile_pool(name="p1ps", bufs=1, space="PSUM") as p1ps,
            ):
                def load_rawt(b):
                    ts = []
                    for j in range(NJ):
                        t = p1.tile([128, S], BF16, tag=f"rawt{j}",
                                    name=f"rawt{j}", bufs=2)
                        nc.sync.dma_start(
                            t[:], rawT_d[b, j * 128:(j + 1) * 128, :])
                        ts.append(t)
                    return ts

                def load_rn(b, sb):
                    t = p1.tile([128, CPB * D], BF16, tag="rn",
                                name="rn", bufs=2)
                    dma_blk(t, rawN_d, b, sb)
                    return t

                # weights first (tiny, needed by first matmuls)
                for j in range(NJ):
                    nc.sync.dma_start(qkT[j][:],
                                      qkT_d[j * 128:(j + 1) * 128, :])
                nc.sync.dma_start(wiB[:], wiB_d[:])
                rawt = load_rawt(0)
                rn_cur = load_rn(0, 0)
                for j in range(NJ):
                    nc.sync.dma_start(woq[j][:],
                                      woq_d[j * 128:(j + 1) * 128, :])
                nc.sync.dma_start(boqT[:], boq_d[:])
                nc.sync.dma_start(wu1B[:], wu1B_d[:])
                nc.sync.dma_start(wu2B[:], wu2B_d[:])

                pdn_pre = [None, None]
                for b in range(BL):
                    Zp = p1.tile([Q, NSB], F32, tag="Zp", bufs=2)
                    A_ps = p1ps.tile([Q, D], F32, tag="A_ps", bufs=2)
                    for sb in range(NSB):
                        # prefetch next block's data
                        if sb + 1 < NSB:
                            rn_nxt = load_rn(b, sb + 1)
                        elif b + 1 < BL:
                            rn_nxt = load_rn(b + 1, 0)
                        else:
                            rn_nxt = None
                        if b == 0 and sb == 0:
                            rawt_nxt = load_rawt(1)
                        if b == 1 and sb == 1:
                            # phase-2 prefetch: b0 pdT + first 2 pdN blocks
                            # (const pool: must outlive the p1 pool scope)
                            for j in range(NJ):
                                nc.sync.dma_start(
                                    pdt0[j][:],
                                    pdT_d[0, j * 128:(j + 1) * 128, :])
                            for k in range(2):
                                t = cp.tile([128, CPB * D], BF16,
                                            tag=f"pdnpre{k}",
                                            name=f"pdnpre{k}")
                                dma_blk(t, pdN_d, 0, k)
                                pdn_pre[k] = t

                        # v-gate pre-pass g = sigmoid(raw . Wi + bi)
                        Gg = p1.tile([128, CPB], F32, tag="Gg", bufs=2)
                        for cc in range(CPB):
                            scr = p1.tile([128, D], BF16, tag="scrb", bufs=2)
                            nc.vector.scalar_tensor_tensor(
                                scr[:], rn_cur[:, cc * D:(cc + 1) * D], 1.0,
                                wiB[:], ALU.mult, ALU.mult,
                                accum_out=Gg[:, cc:cc + 1])
                        nc.scalar.activation(Gg[:], Gg[:], AF.Exp,
                                             scale=-1.0, bias=nbi[:])
                        nc.vector.tensor_scalar_add(Gg[:], Gg[:], 1.0)
                        nc.vector.reciprocal(Gg[:], Gg[:])

                        # scores U = exp(qk @ raw^T)
                        ssl = slice(sb * SBLK, (sb + 1) * SBLK)
                        sc_ps = p1ps.tile([Q, SBLK], F32, tag="sc_ps", bufs=2)
                        for j in range(NJ):
                            nc.tensor.matmul(
                                sc_ps[:], qkT[j][:], rawt[j][:, ssl],
                                start=(j == 0), stop=(j == NJ - 1))
                        U = p1.tile([Q, SBLK], BF16, tag="U", bufs=2)
                        nc.scalar.activation(U[:], sc_ps[:], AF.Exp,
                                             accum_out=Zp[:, sb:sb + 1])
                        # transpose U chunks, fold g in on the way out
                        uts = []
                        for cc in range(CPB):
                            ut_ps = p1ps.tile([128, Q], BF16, tag="ut_ps",
                                              bufs=2)
                            nc.tensor.transpose(
                                ut_ps[:], U[:, cc * 128:(cc + 1) * 128],
                                ident[:Q, :Q])
                            ut = p1.tile([128, Q], BF16, tag="ut", bufs=6)
                            nc.vector.tensor_scalar_mul(
                                ut[:], ut_ps[:], Gg[:, cc:cc + 1])
                            uts.append(ut)
                        for cc in range(CPB):
                            c = sb * CPB + cc
                            for h in range(2):
                                hsl = slice(h * 512, (h + 1) * 512)
                                nc.tensor.matmul(
                                    A_ps[:, hsl], uts[cc][:],
                                    rn_cur[:, cc * D + h * 512:
                                           cc * D + (h + 1) * 512],
                                    start=(c == 0), stop=(c == NCH - 1),
                                    skip_group_check=True)
                        rn_cur = rn_nxt

                    # A_acc += A_ps / (16 * Z)
                    Z1 = p1.tile([Q, 1], F32, tag="Z1", bufs=2)
                    nc.vector.tensor_reduce(Z1[:], Zp[:], mybir.AxisListType.X,
                                            ALU.add)
                    sA = p1.tile([Q, 1], F32, tag="sA", bufs=2)
                    nc.vector.reciprocal(sA[:], Z1[:])
                    nc.vector.tensor_scalar_mul(sA[:], sA[:], 1.0 / B)
                    if b == 0:
                        nc.vector.tensor_scalar_mul(A_acc[:], A_ps[:], sA[:])
                        rawt = rawt_nxt
                    else:
                        nc.vector.scalar_tensor_tensor(
                            A_acc[:], A_ps[:], sA[:], A_acc[:],
                            ALU.mult, ALU.add)

                # aw_local = A_acc . Wu2 rides along in the AllReduce
                nc.vector.scalar_tensor_tensor(
                    scrA[:], A_acc[:], 1.0, wu2B[:],
                    ALU.mult, ALU.mult, accum_out=aw[:])
                nc.gpsimd.dma_start(ar_in[0:Q, 0:D], A_acc[:])
                nc.gpsimd.dma_start(
                    ar_in[Q:Q + 1, 0:Q].rearrange("a b -> b a"), aw[:])

            # ---- AllReduce of partial A across the 8 cores ----
            nc.gpsimd.collective_compute(
                "AllReduce", ALU.add,
                replica_groups=[list(range(NCORES))],
                ins=[ar_in.opt()], outs=[ar_out.opt()],
            )
            nc.gpsimd.dma_start(A_bf[:], ar_out[0:Q, 0:D])
            nc.gpsimd.dma_start(
                awB[:], ar_out[Q:Q + 1, 0:Q].broadcast_to([128, Q]))

            # ================= PHASE 2 =================
            with (
                tc.tile_pool(name="p2", bufs=1) as p2,
                tc.tile_pool(name="p2ps", bufs=1, space="PSUM") as p2ps,
            ):
                def load_pdt(b):
                    ts = []
                    for j in range(NJ):
                        t = p2.tile([128, S], BF16, tag=f"pdt{j}",
                                    name=f"pdt{j}", bufs=1)
                        nc.sync.dma_start(
                            t[:], pdT_d[b, j * 128:(j + 1) * 128, :])
                        ts.append(t)
                    return ts

                def emit_partB(st):
                    (b, sb, Z2, G1, u2s, ut2s, pdn) = st
                    G2 = p2.tile([128, CPB], F32, tag="G2", bufs=2)
                    for cc in range(CPB):
                        scr2 = p2.tile([128, Q], BF16, tag="scr2", bufs=2)
                        nc.vector.scalar_tensor_tensor(
                            scr2[:], u2s[cc][:], 1.0, awB[:],
                            ALU.mult, ALU.mult,
                            accum_out=G2[:, cc:cc + 1])
                    # sc = sigmoid(g1 + g2/Z + cg) / Z
                    rz = p2.tile([128, CPB], F32, tag="rz", bufs=2)
                    nc.vector.reciprocal(rz[:], Z2[:])
                    t4 = p2.tile([128, CPB], F32, tag="t4", bufs=2)
                    nc.vector.tensor_mul(t4[:], G2[:], rz[:])
                    nc.vector.tensor_add(t4[:], t4[:], G1[:])
                    e4 = p2.tile([128, CPB], F32, tag="e4", bufs=2)
                    nc.scalar.activation(e4[:], t4[:], AF.Exp,
                                         scale=-1.0, bias=ncg[:])
                    nc.vector.tensor_scalar_add(e4[:], e4[:], 1.0)
                    nc.vector.reciprocal(e4[:], e4[:])
                    SC = p2.tile([128, CPB], F32, tag="SC", bufs=2)
                    nc.vector.tensor_mul(SC[:], e4[:], rz[:])
                    outt = p2.tile([128, CPB * D], BF16, tag="outt", bufs=2)
                    for cc in range(CPB):
                        for h in range(2):
                            hsl = slice(h * 512, (h + 1) * 512)
                            a2_ps = p2ps.tile([128, 512], F32,
                                              tag="a2_ps", bufs=2)
                            nc.tensor.matmul(a2_ps[:], ut2s[cc][:],
                                             A_bf[:, hsl],
                                             start=True, stop=True)
                            nc.vector.scalar_tensor_tensor(
                                outt[:, cc * D + h * 512:
                                     cc * D + (h + 1) * 512],
                                a2_ps[:], SC[:, cc:cc + 1],
                                pdn[:, cc * D + h * 512:
                                    cc * D + (h + 1) * 512],
                                ALU.mult, ALU.add)
                    dma_blk(outt, out_d, b, sb, store=True)

                blocks = [(b, sb) for b in range(BL) for sb in range(NSB)]
                pdt_cur = pdt0
                pdt_nxt = None
                pdn_queue = []
                pending = []
                for idx, (b, sb) in enumerate(blocks):
                    # prefetch
                    if idx == 0:
                        pdt_nxt = load_pdt(1)
                    if b == 1 and sb == 0:
                        pdt_cur = pdt_nxt
                    if idx + 2 < len(blocks):
                        nb, nsb2 = blocks[idx + 2]
                        pdn_n = p2.tile([128, CPB * D], BF16, tag="pdn",
                                        name="pdn", bufs=5)
                        dma_blk(pdn_n, pdN_d, nb, nsb2)
                        pdn_queue.append(pdn_n)
                    pdn = pdn_pre[idx] if idx < 2 else pdn_queue.pop(0)

                    # ---- partA: s2 + exp + transpose + G1 ----
                    Z2 = p2.tile([128, CPB], F32, tag="Z2", bufs=4)
                    G1 = p2.tile([128, CPB], F32, tag="G1", bufs=4)
                    u2s, ut2s = [], []
                    ssl0 = sb * SBLK
                    for cc in range(CPB):
                        csl = slice(ssl0 + cc * 128, ssl0 + (cc + 1) * 128)
                        s2_ps = p2ps.tile([128, Q], F32, tag="s2_ps",
                                          bufs=2)
                        for j in range(NJ):
                            nc.tensor.matmul(
                                s2_ps[:], pdt_cur[j][:, csl], woq[j][:],
                                start=(j == 0), stop=False,
                                skip_group_check=True)
                        nc.tensor.matmul(
                            s2_ps[:], ones1[:, 0:128], boqT[:],
                            start=False, stop=True, skip_group_check=True)
                        u2 = p2.tile([128, Q], BF16, tag=f"u2{cc}",
                                     name=f"u2{cc}", bufs=4)
                        nc.scalar.activation(u2[:], s2_ps[:], AF.Exp,
                                             accum_out=Z2[:, cc:cc + 1])
                        u2s.append(u2)
                        ut2_ps = p2ps.tile([Q, 128], BF16, tag="ut2_ps",
                                           bufs=2)
                        nc.tensor.transpose(ut2_ps[:], u2[:], ident[:, :])
                        ut2 = p2.tile([Q, 128], BF16, tag=f"ut2{cc}",
                                      name=f"ut2{cc}", bufs=4)
                        nc.scalar.copy(ut2[:], ut2_ps[:])
                        ut2s.append(ut2)
                        scr = p2.tile([128, D], BF16, tag="scrf", bufs=2)
                        nc.vector.scalar_tensor_tensor(
                            scr[:], pdn[:, cc * D:(cc + 1) * D], 1.0,
                            wu1B[:], ALU.mult, ALU.mult,
                            accum_out=G1[:, cc:cc + 1])

                    depth = 3 if idx <= 4 else 2
                    while len(pending) >= depth:
                        emit_partB(pending.pop(0))
                    pending.append((b, sb, Z2, G1, u2s, ut2s, pdn))
                for st in pending:
                    emit_partB(st)

    nc.compile()
    return nc


def _get_prog(bi_v, cgate_v):
    key = (round(bi_v, 9), round(cgate_v, 9))
    if key not in _prog_cache:
        _prog_cache[key] = _build(bi_v, cgate_v)
    return _prog_cache[key]


def kernel(raw, post_dec, mask, questions, Wk, bk, Wi, bi, Wo, bo,
           Wu1, bu1, Wu2, bu2, b1, _trace=False):
    raw = np.asarray(raw, dtype=np.float32)
    post_dec = np.asarray(post_dec, dtype=np.float32)
    questions = np.asarray(questions, dtype=np.float32)
    Wk = np.asarray(Wk, dtype=np.float32)
    Wo = np.asarray(Wo, dtype=np.float32)

    bi_v = float(np.asarray(bi).reshape(-1)[0])
    cgate_v = float(np.asarray(bu1).reshape(-1)[0]
                    + np.asarray(bu2).reshape(-1)[0]
                    + np.asarray(b1).reshape(-1)[0])
    nc = _get_prog(bi_v, cgate_v)

    inv_sqrt_d = np.float32(1.0 / np.sqrt(D))
    inv_sqrt_q = np.float32(1.0 / np.sqrt(Q))
    qk = (questions @ Wk) * inv_sqrt_d                      # [Q, D]
    qkT = np.ascontiguousarray(qk.T).astype(BF)             # [D, Q]
    woqm = (questions @ Wo).T * inv_sqrt_q                  # [D, Q]
    woq = np.ascontiguousarray(woqm).astype(BF)
    boq = np.ascontiguousarray(
        ((questions @ np.asarray(bo, np.float32)) * inv_sqrt_q
         ).reshape(1, Q)).astype(BF)
    wiB = np.ascontiguousarray(
        np.broadcast_to(np.asarray(Wi, np.float32).reshape(1, D), (128, D))
    ).astype(BF)
    wu1B = np.ascontiguousarray(
        np.broadcast_to(np.asarray(Wu1, np.float32).reshape(1, D), (128, D))
    ).astype(BF)
    wu2B = np.ascontiguousarray(
        np.broadcast_to(np.asarray(Wu2, np.float32).reshape(1, D), (Q, D)))

    in_maps = []
    for r in range(NCORES):
        bs = slice(r * BL, (r + 1) * BL)
        rawT = np.ascontiguousarray(
            raw[bs].transpose(0, 2, 1)).astype(BF)
        rawN = np.ascontiguousarray(raw[bs]).astype(BF)
        pdT = np.ascontiguousarray(
            post_dec[bs].transpose(0, 2, 1)).astype(BF)
        pdN = np.ascontiguousarray(post_dec[bs]).astype(BF)
        in_maps.append({
            "rawT": rawT, "rawN": rawN, "pdT": pdT, "pdN": pdN,
            "qkT": qkT, "woq": woq, "boq": boq,
            "wiB": wiB, "wu1B": wu1B, "wu2B": wu2B,
        })

    res = run_bass_kernel_spmd(nc, in_maps, core_ids=list(range(NCORES)),
                               trace=_trace)
    out = np.concatenate(
        [res.results[r]["out"].astype(np.float32) for r in range(NCORES)],
        axis=0)
    if _trace:
        kernel._last_result = res
    return out


# revision 14
# speedup vs baseline: 1.8397x; 1.0620x over previous
"""Trainium2 Bass kernel for nn_Pndb_43344809951805 (scatter_memory).

Data-parallel over batch B=16 across 8 NeuronCores (2 batches/core).

Algebraic rewrites vs the reference:
  Phase 1: scores = (questions @ Wk) @ raw^T  (q.bk bias is softmax-
           invariant over s). Wi is folded in as a 65th stationary
           column, so the v-gate logit row comes free with the scores
           matmul; sigma(g) rides the U transpose and scales the attn
           rows per-partition.
  Phase 2: one [65,512] matmul group per block (stationary = woq chunk
           plus a Wu1 column) yields the read logits transposed and the
           G1 gate row. boq enters as the exp activation's
           per-partition bias.
Scalar engine runs Exp/Copy only (sigmoids via exp to avoid activation
table reloads). DMA is split across both HWDGE rings (sync=SP,
scalar=ACT). HBM-bound: ~42 MB/core.
"""
import sys

sys.path.insert(0, "/opt/trn_rl_repo")

import numpy as np
import ml_dtypes

import concourse.bass as bass
import concourse.bacc as bacc
import concourse.mybir as mybir
import concourse.tile as tile
from concourse import masks
from concourse.bass_utils import run_bass_kernel_spmd

F32 = mybir.dt.float32
BF16 = mybir.dt.bfloat16
AF = mybir.ActivationFunctionType
ALU = mybir.AluOpType
BF = ml_dtypes.bfloat16

B, S, D, Q = 16, 2048, 1024, 64
NCORES = 8
BL = B // NCORES          # local batches per core
SBLK = 512                # s-block
NSB = S // SBLK           # 4 s-blocks per batch
NCH = S // 128            # 16 s-chunks per batch
NJ = D // 128             # 8 contraction chunks
CPB = SBLK // 128         # 4 chunks per s-block
QX = Q + 1                # extra fused gate column/row

_prog_cache = {}


def _build(bi_v: float, cgate_v: float):
    nc = bacc.Bacc("TRN2", target_bir_lowering=False, debug=False,
                   enable_asserts=False, num_devices=NCORES)

    rawT_d = nc.dram_tensor("rawT", [BL, NJ, 128, S], BF16,
                            kind="ExternalInput")
    rawN_d = nc.dram_tensor("rawN", [BL * NCH, 128, D], BF16,
                            kind="ExternalInput")
    pdT_d = nc.dram_tensor("pdT", [BL, NJ, 128, S], BF16,
                           kind="ExternalInput")
    pdN_d = nc.dram_tensor("pdN", [BL * NCH, 128, D], BF16,
                           kind="ExternalInput")
    qkx_d = nc.dram_tensor("qkx", [NJ, 128, QX], BF16, kind="ExternalInput")
    wox_d = nc.dram_tensor("wox", [NJ, 128, QX], BF16, kind="ExternalInput")
    boq_d = nc.dram_tensor("boq", [Q, 1], F32, kind="ExternalInput")
    wu2B_d = nc.dram_tensor("wu2B", [Q, D], F32, kind="ExternalInput")
    out_d = nc.dram_tensor("out", [BL * NCH, 128, D], BF16,
                           kind="ExternalOutput")

    with tile.TileContext(nc) as tc:
        with (
            tc.tile_pool(name="const", bufs=1) as cp,
            tc.tile_pool(name="dram", bufs=1, space="DRAM") as dram,
        ):
            ident = cp.tile([128, 128], BF16, tag="ident")
            masks.make_identity(nc, ident[:])
            nbiB = cp.tile([128, 1], F32, tag="nbiB")
            nc.vector.memset(nbiB[:], -bi_v)
            ncgB = cp.tile([128, 1], F32, tag="ncgB")
            nc.vector.memset(ncgB[:], -cgate_v)

            qkx = cp.tile([128, NJ * QX], BF16, tag="qkx")
            wox = cp.tile([128, NJ * QX], BF16, tag="wox")
            boqc = cp.tile([Q, 1], F32, tag="boqc")
            wu2B = cp.tile([Q, D], F32, tag="wu2B")

            A_acc = cp.tile([Q, D], F32, tag="A_acc")
            A_bf = cp.tile([Q, D], BF16, tag="A_bf")
            awB = cp.tile([128, Q], BF16, tag="awB")
            scrA = cp.tile([Q, D], F32, tag="scrA")
            aw = cp.tile([Q, 1], F32, tag="aw")
            ar_in = dram.tile([Q + 1, D + 8], BF16)
            ar_out = dram.tile([Q + 1, D + 8], BF16)
            awz = cp.tile([Q, 8], BF16, tag="awz")
            nc.vector.memset(awz[:], 0.0)
            zrow = cp.tile([1, D + 8], BF16, tag="zrow")
            nc.vector.memset(zrow[:], 0.0)
            nc.gpsimd.dma_start(ar_in[0:Q, D:D + 8], awz[:])
            nc.gpsimd.dma_start(ar_in[Q:Q + 1, Q:D + 8], zrow[:, Q:D + 8])
            # phase-2 b0 pdT + first pdN blocks prefetched late in phase 1
            pdt0 = cp.tile([128, NJ * S], BF16, tag="pdt0")
            pdn_pre = [cp.tile([128, CPB * D], BF16, tag=f"pdnpre{k}",
                               name=f"pdnpre{k}")
                       for k in range(2)]

            def load_batchT(tile_, dram_t, b, s0=0, s1=S):
                nc.sync.dma_start(
                    tile_[:, :].rearrange("p (j s) -> p j s", j=NJ)
                    [:, :, s0:s1],
                    dram_t[b].rearrange("j p s -> p j s")[:, :, s0:s1])

            def load_n(tile_, dram_t, b, sb, eng):
                c0 = b * NCH + sb * CPB
                eng.dma_start(
                    tile_[:].rearrange("p (c d) -> p c d", c=CPB),
                    dram_t[c0:c0 + CPB].rearrange("c p d -> p c d"))

            # ================= PHASE 1 =================
            with (
                tc.tile_pool(name="p1", bufs=1) as p1,
                tc.tile_pool(name="p1ps", bufs=1, space="PSUM") as p1ps,
            ):
                def load_rn(b, sb):
                    t = p1.tile([128, CPB * D], BF16, tag="rn",
                                name="rn", bufs=2)
                    load_n(t, rawN_d, b, sb, nc.scalar)
                    return t

                # weights first (tiny, needed by first matmuls)
                nc.sync.dma_start(
                    qkx[:].rearrange("p (j c) -> p j c", j=NJ),
                    qkx_d.rearrange("j p c -> p j c"))
                rawt = p1.tile([128, NJ * S], BF16, tag="rawt0")
                load_batchT(rawt, rawT_d, 0, 0, SBLK)
                rn_cur = load_rn(0, 0)
                load_batchT(rawt, rawT_d, 0, SBLK, S)
                nc.sync.dma_start(
                    wox[:].rearrange("p (j c) -> p j c", j=NJ),
                    wox_d.rearrange("j p c -> p j c"))
                nc.sync.dma_start(boqc[:], boq_d[:])
                nc.sync.dma_start(wu2B[:], wu2B_d[:])

                for b in range(BL):
                    Zp = p1.tile([Q, NSB], F32, tag="Zp", bufs=2)
                    A_ps = p1ps.tile([Q, D], F32, tag="A_ps", bufs=2)
                    for sb in range(NSB):
                        # prefetch next block's data
                        if sb + 1 < NSB:
                            rn_nxt = load_rn(b, sb + 1)
                        elif b + 1 < BL:
                            rn_nxt = load_rn(b + 1, 0)
                        else:
                            rn_nxt = None
                        if b == 0 and sb == 0:
                            rawt_nxt = p1.tile([128, NJ * S], BF16,
                                               tag="rawt1")
                            load_batchT(rawt_nxt, rawT_d, 1)
                        if b == 1 and sb == 1:
                            load_batchT(pdt0, pdT_d, 0)
                            load_n(pdn_pre[0], pdN_d, 0, 0, nc.scalar)
                            load_n(pdn_pre[1], pdN_d, 0, 1, nc.scalar)

                        # scores U[0:64] = exp(qk @ raw^T);
                        # row 64 = exp(-(raw.Wi + bi)) for the v-gate
                        sc_ps = p1ps.tile([QX, SBLK], F32, tag="sc_ps",
                                          bufs=2)
                        for j in range(NJ):
                            nc.tensor.matmul(
                                sc_ps[:], qkx[:, j * QX:(j + 1) * QX],
                                rawt[:, j * S + sb * SBLK:
                                     j * S + (sb + 1) * SBLK],
                                start=(j == 0), stop=(j == NJ - 1))
                        U = p1.tile([QX, SBLK], BF16, tag="U", bufs=2)
                        nc.scalar.activation(U[0:Q, :], sc_ps[0:Q, :],
                                             AF.Exp,
                                             accum_out=Zp[:, sb:sb + 1])
                        nc.scalar.activation(U[Q:QX, :], sc_ps[Q:QX, :],
                                             AF.Exp, scale=-1.0,
                                             bias=nbiB[0:1, :])
                        # transpose U chunks; fold g in on the way out
                        uts = []
                        for cc in range(CPB):
                            ut_ps = p1ps.tile([128, QX], BF16, tag="ut_ps",
                                              bufs=2)
                            nc.tensor.transpose(
                                ut_ps[:], U[:, cc * 128:(cc + 1) * 128],
                                ident[:QX, :QX])
                            gcol = p1.tile([128, 1], F32, tag="gcol",
                                           bufs=4)
                            nc.vector.tensor_scalar_add(
                                gcol[:], ut_ps[:, Q:QX], 1.0)
                            nc.vector.reciprocal(gcol[:], gcol[:])
                            ut = p1.tile([128, Q], BF16, tag="ut", bufs=6)
                            nc.vector.tensor_scalar_mul(
                                ut[:], ut_ps[:, 0:Q], gcol[:])
                            uts.append(ut)
                        for cc in range(CPB):
                            c = sb * CPB + cc
                            for h in range(2):
                                nc.tensor.matmul(
                                    A_ps[:, h * 512:(h + 1) * 512],
                                    uts[cc][:],
                                    rn_cur[:, cc * D + h * 512:
                                           cc * D + (h + 1) * 512],
                                    start=(c == 0), stop=(c == NCH - 1),
                                    skip_group_check=True)
                        rn_cur = rn_nxt

                    # A_acc += A_ps / (16 * Z)
                    Z1 = p1.tile([Q, 1], F32, tag="Z1", bufs=2)
                    nc.vector.tensor_reduce(Z1[:], Zp[:], mybir.AxisListType.X,
                                            ALU.add)
                    sA = p1.tile([Q, 1], F32, tag="sA", bufs=2)
                    nc.vector.reciprocal(sA[:], Z1[:])
                    nc.vector.tensor_scalar_mul(sA[:], sA[:], 1.0 / B)
                    if b == 0:
                        nc.vector.tensor_scalar_mul(A_acc[:], A_ps[:], sA[:])
                        rawt = rawt_nxt
                    else:
                        nc.vector.scalar_tensor_tensor(
                            A_acc[:], A_ps[:], sA[:], A_acc[:],
                            ALU.mult, ALU.add)

                # aw_local = A_acc . Wu2 rides along in the AllReduce
                nc.vector.scalar_tensor_tensor(
                    scrA[:], A_acc[:], 1.0, wu2B[:],
                    ALU.mult, ALU.mult, accum_out=aw[:])
                nc.gpsimd.dma_start(ar_in[0:Q, 0:D], A_acc[:])
                nc.gpsimd.dma_start(
                    ar_in[Q:Q + 1, 0:Q].rearrange("a b -> b a"), aw[:])

            # ---- AllReduce of partial A across the 8 cores ----
            nc.gpsimd.collective_compute(
                "AllReduce", ALU.add,
                replica_groups=[list(range(NCORES))],
                ins=[ar_in.opt()], outs=[ar_out.opt()],
            )
            nc.gpsimd.dma_start(A_bf[:], ar_out[0:Q, 0:D])
            nc.gpsimd.dma_start(
                awB[:], ar_out[Q:Q + 1, 0:Q].broadcast_to([128, Q]))

            # ================= PHASE 2 =================
            with (
                tc.tile_pool(name="p2", bufs=1) as p2,
                tc.tile_pool(name="p2ps", bufs=1, space="PSUM") as p2ps,
            ):
                def emit_partB(st):
                    (b, sb, Z2, G1, ut2x, u2s, pdn) = st
                    G2 = p2.tile([128, CPB], F32, tag="G2", bufs=2)
                    for cc in range(CPB):
                        scr2 = p2.tile([128, Q], BF16, tag="scr2", bufs=2)
                        nc.vector.scalar_tensor_tensor(
                            scr2[:], u2s[cc][:], 1.0, awB[:],
                            ALU.mult, ALU.mult,
                            accum_out=G2[:, cc:cc + 1])
                    # SC = sigmoid(G1 + G2/Z2 + cg) / Z2  (exp-form)
                    rz = p2.tile([128, CPB], F32, tag="rz", bufs=2)
                    nc.vector.reciprocal(rz[:], Z2[:])
                    t4 = p2.tile([128, CPB], F32, tag="t4", bufs=2)
                    nc.vector.tensor_mul(t4[:], G2[:], rz[:])
                    nc.vector.tensor_add(t4[:], t4[:], G1[:])
                    e4 = p2.tile([128, CPB], F32, tag="e4", bufs=2)
                    nc.scalar.activation(e4[:], t4[:], AF.Exp,
                                         scale=-1.0, bias=ncgB[:])
                    nc.vector.tensor_scalar_add(e4[:], e4[:], 1.0)
                    nc.vector.reciprocal(e4[:], e4[:])
                    SC = p2.tile([128, CPB], F32, tag="SC", bufs=2)
                    nc.vector.tensor_mul(SC[:], e4[:], rz[:])
                    outt = p2.tile([128, CPB * D], BF16, tag="outt", bufs=3)
                    for cc in range(CPB):
                        eng = nc.vector
                        for h in range(2):
                            a2_ps = p2ps.tile([128, 512], F32, tag="a2_ps",
                                              bufs=3)
                            nc.tensor.matmul(
                                a2_ps[:],
                                ut2x[0:Q, cc * 128:(cc + 1) * 128],
                                A_bf[:, h * 512:(h + 1) * 512],
                                start=True, stop=True)
                            eng.scalar_tensor_tensor(
                                outt[:, cc * D + h * 512:
                                     cc * D + (h + 1) * 512],
                                a2_ps[:], SC[:, cc:cc + 1],
                                pdn[:, cc * D + h * 512:
                                    cc * D + (h + 1) * 512],
                                ALU.mult, ALU.add)
                    c0 = b * NCH + sb * CPB
                    nc.scalar.dma_start(
                        out_d[c0:c0 + CPB].rearrange("c p d -> p c d"),
                        outt[:].rearrange("p (c d) -> p c d", c=CPB))

                blocks = [(b, sb) for b in range(BL) for sb in range(NSB)]
                pdt_cur = pdt0
                pdt_nxt = None
                pdn_queue = []
                pending = []
                for idx, (b, sb) in enumerate(blocks):
                    if idx == 0:
                        pdt_nxt = p2.tile([128, NJ * S], BF16, tag="pdt1")
                        load_batchT(pdt_nxt, pdT_d, 1)
                    if b == 1 and sb == 0:
                        pdt_cur = pdt_nxt
                    if idx + 2 < len(blocks):
                        nb, nsb2 = blocks[idx + 2]
                        pdn_n = p2.tile([128, CPB * D], BF16, tag="pdn",
                                        name="pdn", bufs=6)
                        load_n(pdn_n, pdN_d, nb, nsb2, nc.sync)
                        pdn_queue.append(pdn_n)
                    pdn = pdn_pre[idx] if idx < 2 else pdn_queue.pop(0)

                    # ---- partA: s2T + exp + transpose(+G1) ----
                    s2t_ps = p2ps.tile([QX, SBLK], F32, tag="s2t_ps",
                                       bufs=2)
                    for j in range(NJ):
                        nc.tensor.matmul(
                            s2t_ps[:], wox[:, j * QX:(j + 1) * QX],
                            pdt_cur[:, j * S + sb * SBLK:
                                    j * S + (sb + 1) * SBLK],
                            start=(j == 0), stop=(j == NJ - 1))
                    ut2x = p2.tile([QX, SBLK], BF16, tag="ut2x", bufs=6)
                    nc.scalar.activation(ut2x[0:Q, :], s2t_ps[0:Q, :],
                                         AF.Exp, bias=boqc[:])
                    nc.scalar.copy(ut2x[Q:QX, :], s2t_ps[Q:QX, :])
                    Z2 = p2.tile([128, CPB], F32, tag="Z2", bufs=5)
                    G1 = p2.tile([128, CPB], F32, tag="G1", bufs=5)
                    u2s = []
                    for cc in range(CPB):
                        u2c_ps = p2ps.tile([128, QX], BF16, tag="u2c_ps",
                                           bufs=2)
                        nc.tensor.transpose(
                            u2c_ps[:], ut2x[:, cc * 128:(cc + 1) * 128],
                            ident[:QX, :QX])
                        u2 = p2.tile([128, Q], BF16, tag=f"u2_{cc}",
                                     name=f"u2_{cc}", bufs=5)
                        nc.scalar.activation(u2[:], u2c_ps[:, 0:Q],
                                             AF.Copy,
                                             accum_out=Z2[:, cc:cc + 1])
                        nc.scalar.copy(G1[:, cc:cc + 1], u2c_ps[:, Q:QX])
                        u2s.append(u2)

                    depth = 4 if idx <= 4 else 2
                    while len(pending) >= depth:
                        emit_partB(pending.pop(0))
                    pending.append((b, sb, Z2, G1, ut2x, u2s, pdn))
                for st in pending:
                    emit_partB(st)

    nc.compile()
    return nc


def _get_prog(bi_v, cgate_v):
    key = (round(bi_v, 9), round(cgate_v, 9))
    if key not in _prog_cache:
        _prog_cache[key] = _build(bi_v, cgate_v)
    return _prog_cache[key]


def kernel(raw, post_dec, mask, questions, Wk, bk, Wi, bi, Wo, bo,
           Wu1, bu1, Wu2, bu2, b1, _trace=False):
    raw = np.asarray(raw, dtype=np.float32)
    post_dec = np.asarray(post_dec, dtype=np.float32)
    questions = np.asarray(questions, dtype=np.float32)
    Wk = np.asarray(Wk, dtype=np.float32)
    Wo = np.asarray(Wo, dtype=np.float32)

    bi_v = float(np.asarray(bi).reshape(-1)[0])
    cgate_v = float(np.asarray(bu1).reshape(-1)[0]
                    + np.asarray(bu2).reshape(-1)[0]
                    + np.asarray(b1).reshape(-1)[0])
    nc = _get_prog(bi_v, cgate_v)

    inv_sqrt_d = np.float32(1.0 / np.sqrt(D))
    inv_sqrt_q = np.float32(1.0 / np.sqrt(Q))
    # stationaries with the fused gate column
    qkx = np.empty((D, QX), np.float32)
    qkx[:, 0:Q] = (questions @ Wk).T * inv_sqrt_d
    qkx[:, Q] = np.asarray(Wi, np.float32).reshape(D)
    wox = np.empty((D, QX), np.float32)
    wox[:, 0:Q] = (questions @ Wo).T * inv_sqrt_q
    wox[:, Q] = np.asarray(Wu1, np.float32).reshape(D)
    qkx = np.ascontiguousarray(qkx.reshape(NJ, 128, QX)).astype(BF)
    wox = np.ascontiguousarray(wox.reshape(NJ, 128, QX)).astype(BF)
    boq = np.ascontiguousarray(
        ((questions @ np.asarray(bo, np.float32)) * inv_sqrt_q
         ).reshape(Q, 1)).astype(np.float32)
    wu2B = np.ascontiguousarray(
        np.broadcast_to(np.asarray(Wu2, np.float32).reshape(1, D), (Q, D)))

    in_maps = []
    for r in range(NCORES):
        bs = slice(r * BL, (r + 1) * BL)
        rawT = np.ascontiguousarray(
            raw[bs].transpose(0, 2, 1)).astype(BF).reshape(BL, NJ, 128, S)
        rawN = np.ascontiguousarray(raw[bs]).astype(BF).reshape(
            BL * NCH, 128, D)
        pdT = np.ascontiguousarray(
            post_dec[bs].transpose(0, 2, 1)).astype(BF).reshape(
            BL, NJ, 128, S)
        pdN = np.ascontiguousarray(post_dec[bs]).astype(BF).reshape(
            BL * NCH, 128, D)
        in_maps.append({
            "rawT": rawT, "rawN": rawN, "pdT": pdT, "pdN": pdN,
            "qkx": qkx, "wox": wox, "boq": boq, "wu2B": wu2B,
        })

    res = run_bass_kernel_spmd(nc, in_maps, core_ids=list(range(NCORES)),
                               trace=_trace)
    out = np.concatenate(
        [res.results[r]["out"].astype(np.float32).reshape(BL, S, D)
         for r in range(NCORES)],
        axis=0)
    if _trace:
        kernel._last_result = res
    return out
